# revision 2
# baseline (speedup 1.0000x reference)
"""Differential attention kernel for 8 Trainium2 NeuronCores (v3, transposed).

Sharding: core c handles batch b = c//4, query rows [(c%4)*1024, (c%4+1)*1024).
Each core receives x[b]^T (bf16, key-columns rolled so its own query block is
first), computes K^T/Q^T projections W-stationary, V x-stationary, applies
RoPE on the transposed layout.

Scores are computed TRANSPOSED (keys on partitions): A^T[k,q] chunks land in
PSUM as [128k, 128q] tiles, exp runs on the scalar engine with NO accum_out,
and the resulting E^T chunks feed P@V directly as the stationary operand —
no DMA XBAR transposes, no P-combine passes.  Row sums d1/d2 come free from
a ones-column appended to V (PV rhs is 132 wide); the differential combine
collapses to two per-row scalings of the two [128,128] branch PV outputs:
O = r1*O1 - lam*r2*O2.

Schedule: groups of 2 q-tiles; inside a group quarter-rounds h0..h3 with the
PV chunk batch of round h emitted at the end of round h (or once its V
quarter exists), so the tensor engine never waits on exp latency.  K1-3 and
V0-3 projections are interleaved into the early rounds' seams.
"""

import math
from contextlib import ExitStack

import ml_dtypes
import numpy as np

import concourse.bass as bass
import concourse.mybir as mybir
import concourse.tile as tile
from concourse import bacc
from concourse.bass_utils import run_bass_kernel_spmd

B, S, D = 2, 4096, 1024
HD = 64
ROT = 128
NQ = 1024  # query rows per core
N_CORES = 8
DC = D // 128  # contraction chunks for the projections
KQ = 1024  # keys per quarter
NKQ = S // KQ  # 4 quarters
NMAX = 512  # moving-operand cap
VW = 132  # PV rhs width: 128 V cols + ones col + 3 pad
FP32 = mybir.dt.float32
BF16 = mybir.dt.bfloat16
AF = mybir.ActivationFunctionType
ALU = mybir.AluOpType

GROUPS = [[0, 1], [2, 3], [4, 5], [6, 7]]

DEBUG_DUMPS = False

_prog_cache = {}


def _build_kernel(ctx: ExitStack, tc, xT, w_sb_aps, cosT, sinT, lamn, out, dbg=None):
    nc = tc.nc

    const = ctx.enter_context(tc.tile_pool(name="const", bufs=1))
    xt_pool = ctx.enter_context(tc.tile_pool(name="xt", bufs=4))

    xTr = xT.rearrange("(a p) s -> p a s", p=128)
    w_sb = {}
    cos_sb = const.tile([128, S], BF16, tag="cos")
    sin_sb = const.tile([128, S], BF16, tag="sin")
    lam_sb = const.tile([128, 1], FP32, tag="lam")
    kT = [const.tile([128, KQ], BF16, tag=f"kT{h}", name=f"kT{h}") for h in range(NKQ)]
    qT = const.tile([128, NQ], BF16, tag="qT")
    # V chunks [128k, 132]: 128 V cols, a ones column for the row sums, 3 pad
    v2 = const.tile([128, S // 128, VW], BF16, tag="v2")

    xt_q = [None] * NKQ
    xt_q[0] = xt_pool.tile([128, DC, KQ], BF16, tag="xt", name="xt0")

    # --- input DMAs, ordered so the Q/K0 path starts ASAP ---
    def load_w(name):
        t = const.tile([128, DC, ROT], BF16, tag=name, name=name)
        nc.sync.dma_start(t[:], w_sb_aps[name].rearrange("(a p) r -> p a r", p=128))
        w_sb[name] = t

    load_w("wq")
    # first quarter arrives in dc-chunks so the Q projection starts after
    # ~1/8 of the transfer instead of all of it
    for dc in range(DC):
        nc.sync.dma_start(xt_q[0][:, dc : dc + 1, :], xTr[:, dc : dc + 1, 0:KQ])
        if dc == 3:
            nc.sync.dma_start(cos_sb[:, 0:512], cosT[:, 0:512])
            nc.sync.dma_start(sin_sb[:, 0:512], sinT[:, 0:512])
    nc.sync.dma_start(cos_sb[:, 512:1024], cosT[:, 512:1024])
    nc.sync.dma_start(sin_sb[:, 512:1024], sinT[:, 512:1024])
    nc.sync.dma_start(lam_sb[:], lamn)
    load_w("wk")
    for h in range(1, NKQ):
        xt_q[h] = xt_pool.tile([128, DC, KQ], BF16, tag="xt", name=f"xt{h}")
        ksl = slice(h * KQ, (h + 1) * KQ)
        nc.sync.dma_start(xt_q[h][:], xTr[:, :, ksl])
        nc.sync.dma_start(cos_sb[:, ksl], cosT[:, ksl])
        nc.sync.dma_start(sin_sb[:, ksl], sinT[:, ksl])
    load_w("wv")

    # ones / pad columns of the V chunks
    nc.gpsimd.memset(v2[:, :, 128:129], 1.0)
    nc.gpsimd.memset(v2[:, :, 129:VW], 0.0)

    # PSUM: 3 rotating [128,1024] score slabs (6 banks) + 2 O banks = 8
    ps = ctx.enter_context(tc.tile_pool(name="ps", bufs=1, space="PSUM"))
    pso = ctx.enter_context(tc.tile_pool(name="pso", bufs=1, space="PSUM"))
    ropetmp = ctx.enter_context(tc.tile_pool(name="ropetmp", bufs=2))
    et_pool = ctx.enter_context(tc.tile_pool(name="et", bufs=5))
    dstat = ctx.enter_context(tc.tile_pool(name="dstat", bufs=2))
    work = ctx.enter_context(tc.tile_pool(name="work", bufs=2))

    sidx = [0]

    def salloc(name, shape=None):
        t = ps.tile(shape or [128, KQ], FP32, tag=f"s{sidx[0] % 3}", name=name)
        sidx[0] += 1
        return t

    # prime the Exp table on the scalar engine before the critical stream
    warm = dstat.tile([128, 1], FP32, tag="warm")
    nc.scalar.activation(warm[:], lam_sb[:], AF.Exp)

    def rope(dst, src_ps, s_off, width):
        # bf16 intermediates halve DVE element cost; all on DVE (Pool's 0.42
        # tensor-op efficiency would sit on the K-ready latency chain)
        csl = slice(s_off, s_off + width)
        t1 = ropetmp.tile([128, KQ], BF16, tag="t1")
        t2 = ropetmp.tile([128, KQ], BF16, tag="t2")
        nc.vector.tensor_mul(t1[:, 0:width], src_ps[:], cos_sb[:, csl])
        nc.vector.tensor_mul(t2[0:64, 0:width], src_ps[64:128, :], sin_sb[0:64, csl])
        nc.vector.tensor_mul(t2[64:128, 0:width], src_ps[0:64, :], sin_sb[64:128, csl])
        nc.vector.tensor_add(dst, t1[:, 0:width], t2[:, 0:width])

    def proj_cols(dst_ps, w_tile, xq, c0, width):
        done = 0
        while done < width:
            w = min(NMAX, width - done)
            for dc in range(DC):
                nc.tensor.matmul(
                    dst_ps[:, done : done + w],
                    lhsT=w_tile[:, dc, :],
                    rhs=xq[:, dc, c0 + done : c0 + done + w],
                    start=(dc == 0),
                    stop=(dc == DC - 1),
                )
            done += w

    def kqproj(h, w_name, dst, half):
        """One 512-column half of a K/Q quarter projection + rope."""
        pk = salloc(f"p{w_name}{h}_{half}")
        c0 = half * 512
        proj_cols(pk, w_sb[w_name], xt_q[h], c0, 512)
        rope(dst[:, c0 : c0 + 512], pk[:, 0:512], h * KQ + c0, 512)

    def vproj_quarter(h):
        # 8 key-chunks of 128 rows into one slab, drained into the strided
        # v2 chunk layout by one Pool copy
        pv = salloc(f"pv{h}")
        for c in range(8):
            so = c * 128
            for dc in range(DC):
                nc.tensor.matmul(
                    pv[:, so : so + 128],
                    lhsT=xt_q[h][:, dc, so : so + 128],
                    rhs=w_sb["wv"][:, dc, :],
                    start=(dc == 0),
                    stop=(dc == DC - 1),
                )
        # DVE drain (GPSIMD cannot access PSUM)
        nc.vector.tensor_copy(
            v2[:, h * 8 : (h + 1) * 8, 0:128], pv[:].rearrange("p (c f) -> p c f", f=128)
        )

    ets = {}
    o_ps = {}

    def scT_exp(qt, br, h):
        """Transposed scores + exp for one (q-tile, branch, key-quarter)."""
        qsl = slice(qt * 128, (qt + 1) * 128)
        lo, hi = (0, 64) if br == 0 else (64, 128)
        psc = salloc(f"psc{qt}_{h}_{br}")
        pscr = psc[:].rearrange("p (c f) -> p c f", f=128)
        for c in range(8):
            nc.tensor.matmul(
                pscr[:, c, :],
                lhsT=kT[h][lo:hi, c * 128 : c * 128 + 128],
                rhs=qT[lo:hi, qsl],
                start=True,
                stop=True,
                tile_position=(br * 64, 0),
            )
        et = et_pool.tile(
            [128, 8, 128], BF16, tag=f"et{br}_{qt % 2}", name=f"et{br}_{qt}_{h}"
        )
        nc.scalar.activation(et[:], pscr[:], AF.Exp, scale=HD**-0.5)
        if dbg is not None and qt == 0 and h == 0:
            nc.sync.dma_start(dbg[f"e{br}"], et[:])
        ets[qt, br, h] = et

    def pv_chunks(qt, h):
        """One quarter of the E^T@V accumulation, both branches.

        Each branch accumulates in its OWN psum bank: PSUM tolerates exactly
        one open accumulation group per bank — two interleaved start/stop
        groups sharing a bank corrupt each other.
        """
        for br in range(2):
            if (qt, br) not in o_ps:
                o_ps[qt, br] = pso.tile(
                    [128, VW], FP32, tag=f"o{br}", name=f"o{br}_{qt}"
                )
            o_t = o_ps[qt, br]
            et = ets.pop((qt, br, h))
            for c in range(8):
                kc = h * 8 + c
                nc.tensor.matmul(
                    o_t[:],
                    lhsT=et[:, c, :],
                    rhs=v2[:, kc, :],
                    start=(kc == 0),
                    stop=(kc == S // 128 - 1),
                )

    def finish_qt(qt):
        """O = r1*O1 - lam*r2*O2 from the two branch PV outputs + store."""
        o1_t = o_ps.pop((qt, 0))
        o2_t = o_ps.pop((qt, 1))
        if dbg is not None and qt == 0:
            ocp = work.tile([128, 2 * VW], FP32, tag="dbgo")
            nc.vector.tensor_copy(ocp[:, 0:VW], o1_t[:])
            nc.vector.tensor_copy(ocp[:, VW : 2 * VW], o2_t[:])
            nc.sync.dma_start(dbg["o"], ocp[:])
        r1 = dstat.tile([128, 1], FP32, tag="r1", name=f"r1_{qt}")
        nc.vector.reciprocal(r1[:], o1_t[:, 128:129])
        r2 = dstat.tile([128, 1], FP32, tag="r2", name=f"r2_{qt}")
        nc.vector.reciprocal(r2[:], o2_t[:, 128:129])
        s2 = dstat.tile([128, 1], FP32, tag="s2", name=f"s2_{qt}")
        nc.vector.tensor_mul(s2[:], r2[:], lam_sb[:])
        o1 = work.tile([128, ROT], FP32, tag="o1", name=f"o1_{qt}")
        nc.vector.tensor_scalar_mul(o1[:], o1_t[:, 0:128], r1[:])
        o2 = work.tile([128, ROT], FP32, tag="o2", name=f"o2_{qt}")
        nc.vector.scalar_tensor_tensor(
            o2[:], o2_t[:, 0:128], s2[:], o1[:], ALU.mult, ALU.add
        )
        # SWDGE path keeps the small output stores off the HWDGE xbar
        nc.gpsimd.dma_start(out[qt * 128 : (qt + 1) * 128, :], o2[:])

    # ---- schedule ----
    # Q-half0 (covers q-tiles 0-3) + K0 only, then attention starts;
    # K1-half0 fills the rope-wait bubble before the first scores
    kqproj(0, "wq", qT, 0)
    kqproj(0, "wk", kT[0], 0)
    kqproj(0, "wk", kT[0], 1)
    kqproj(1, "wk", kT[1], 0)

    # extras per (qt, h) seam: K1-3/V0-3/Q-half1 projections spread so the
    # gathered K quarter h is ready before round h and V quarter h before
    # the pv batch of h
    extras = {
        (0, 0): [lambda: kqproj(1, "wk", kT[1], 1)],
        (0, 1): [lambda: kqproj(2, "wk", kT[2], 0),
                 lambda: kqproj(2, "wk", kT[2], 1)],
        (0, 2): [lambda: kqproj(3, "wk", kT[3], 0),
                 lambda: kqproj(3, "wk", kT[3], 1)],
        (0, 3): [lambda: vproj_quarter(0)],
        (1, 0): [lambda: vproj_quarter(1)],
        (1, 1): [lambda: vproj_quarter(2)],
        (1, 2): [lambda: vproj_quarter(3)],
        (1, 3): [lambda: kqproj(0, "wq", qT, 1)],
    }
    v_seam = {0: (0, 3), 1: (1, 0), 2: (1, 1), 3: (1, 2)}

    pending = []
    for qt in range(8):
        for h in range(NKQ):
            for br in range(2):
                scT_exp(qt, br, h)
            for ex in extras.get((qt, h), []):
                ex()
            # drain pending PV batches in strict FIFO order (skipping would
            # deadlock the in-order PE queue through the single-buffer O
            # banks); stop at the first V-gated batch, at most 2 per seam,
            # never the round just emitted
            emitted = 0
            while pending and emitted < 2:
                bq, bh = pending[0]
                if (bq, bh) == (qt, h) or (qt, h) < v_seam[bh]:
                    break
                pv_chunks(bq, bh)
                if bh == NKQ - 1:
                    finish_qt(bq)
                pending.pop(0)
                emitted += 1
            pending.append((qt, h))
    for bq, bh in pending:
        pv_chunks(bq, bh)
        if bh == NKQ - 1:
            finish_qt(bq)


def _get_program(repeat=1):
    if repeat in _prog_cache:
        return _prog_cache[repeat]
    nc = bacc.Bacc("TRN2", target_bir_lowering=False, debug=False, num_devices=N_CORES)
    xT = nc.dram_tensor("xT", [D, S], BF16, kind="ExternalInput").ap()
    wq = nc.dram_tensor("wq", [D, ROT], BF16, kind="ExternalInput").ap()
    wk = nc.dram_tensor("wk", [D, ROT], BF16, kind="ExternalInput").ap()
    wv = nc.dram_tensor("wv", [D, ROT], BF16, kind="ExternalInput").ap()
    cosT = nc.dram_tensor("cosT", [ROT, S], BF16, kind="ExternalInput").ap()
    sinT = nc.dram_tensor("sinT", [ROT, S], BF16, kind="ExternalInput").ap()
    lamn = nc.dram_tensor("lamn", [128, 1], FP32, kind="ExternalInput").ap()
    out = nc.dram_tensor("out", [NQ, ROT], FP32, kind="ExternalOutput").ap()
    dbg = None
    if DEBUG_DUMPS:
        dbg = {
            "o": nc.dram_tensor("dbg_o", [128, 2 * VW], FP32, kind="ExternalOutput").ap(),
            "e0": nc.dram_tensor("dbg_e0", [128, 8, 128], BF16, kind="ExternalOutput").ap(),
            "e1": nc.dram_tensor("dbg_e1", [128, 8, 128], BF16, kind="ExternalOutput").ap(),
        }

    with tile.TileContext(nc) as tc:
        for rep in range(repeat):
            if rep > 0:
                # isolate repeated bodies so timing slopes measure single-run latency
                tc.strict_bb_all_engine_barrier()
            with ExitStack() as ctx:
                _build_kernel(
                    ctx, tc, xT, {"wq": wq, "wk": wk, "wv": wv}, cosT, sinT, lamn, out,
                    dbg=dbg,
                )
    nc.compile()
    _prog_cache[repeat] = nc
    return nc


def make_in_maps(x, Wq, Wk, Wv, lambda_q1, lambda_q2, lambda_k1, lambda_k2):
    x = np.asarray(x, dtype=np.float32)
    Wq, Wk, Wv = (np.asarray(w, dtype=np.float32) for w in (Wq, Wk, Wv))
    lq1, lq2, lk1, lk2 = (
        np.asarray(v, dtype=np.float32)
        for v in (lambda_q1, lambda_q2, lambda_k1, lambda_k2)
    )

    lam_init = 0.8 - 0.6 * math.exp(-0.3 * 1)
    lam = float(
        np.exp(np.sum(lq1 * lk1)) - np.exp(np.sum(lq2 * lk2)) + lam_init
    )

    inv = 1.0 / (10000.0 ** (np.arange(0, ROT, 2, dtype=np.float32) / ROT))
    freqs = np.arange(S, dtype=np.float32)[:, None] * inv[None, :]  # [S, 64]
    cosh = np.cos(freqs)
    sinh = np.sin(freqs)
    cosT_full = np.concatenate([cosh, cosh], axis=1).T  # [128, S]
    sinT_full = np.concatenate([-sinh, sinh], axis=1).T

    bf = ml_dtypes.bfloat16
    wq_b, wk_b, wv_b = (np.ascontiguousarray(w, dtype=bf) for w in (Wq, Wk, Wv))
    lam_arr = np.full((128, 1), -lam, dtype=np.float32)

    in_maps = []
    for c in range(N_CORES):
        b, qoff = c // 4, (c % 4) * NQ
        xTr = np.roll(x[b].T, -qoff, axis=1)
        in_maps.append(
            {
                "xT": np.ascontiguousarray(xTr, dtype=bf),
                "wq": wq_b, "wk": wk_b, "wv": wv_b,
                "cosT": np.ascontiguousarray(np.roll(cosT_full, -qoff, axis=1), dtype=bf),
                "sinT": np.ascontiguousarray(np.roll(sinT_full, -qoff, axis=1), dtype=bf),
                "lamn": lam_arr,
            }
        )
    return in_maps


def assemble_out(results):
    outs = [np.asarray(results[c]["out"], dtype=np.float32) for c in range(N_CORES)]
    return np.stack(
        [np.concatenate(outs[0:4], axis=0), np.concatenate(outs[4:8], axis=0)]
    )


def kernel(x, Wq, Wk, Wv, lambda_q1, lambda_q2, lambda_k1, lambda_k2):
    in_maps = make_in_maps(x, Wq, Wk, Wv, lambda_q1, lambda_q2, lambda_k1, lambda_k2)
    nc = _get_program()
    res = run_bass_kernel_spmd(nc, in_maps, list(range(N_CORES)))
    return assemble_out(res.results)
